# revision 49
# baseline (speedup 1.0000x reference)
"""Differential multi-head attention kernel for Trainium2 (8 NeuronCores).

Data-parallel over batch (16/8 = 2 per core). Per core, software-pipelined:

  init:  weights cast to bf16 once (Wq/Wk to DRAM scratch in a head-paired
         layout: head h's stationary cols are [q1|q2], so the dual-softmax
         score matmuls row-pack into PE array halves 0:64 / 64:128 and run
         concurrently). Wq loads go first so batch-0 Q-proj starts early.
  P1(b): x -> bf16 -> PE transpose -> xT; Q/K projections write Q12/K12
         (head h: side1 on partitions 0:64, side2 on 64:128); V -> vaug
         (ones col 64 makes the softmax denominators fall out of ctx MMs).
  P2(b): per head: row-packed score MMs into a merged [128,2,1024] psum,
         ONE exp per (h,kp) for both sides (ACT, bf16 out), ctx MMs
         accumulate [65, S]; ctx drained on DVE (row 64 = denominators,
         gathered into 32-aligned quadrants of den_all).
  tail(b), per 6-head group: reciprocals, -lam fold, bf16 broadcast,
         combines on DVE (stats via accum_out), per-group GroupNorm
         (rstd = exp(-0.5 ln(var+eps)) keeps ACT on one table set),
         apply, per-group scratch write.
  P3(b): read the bf16 scratch reinterpreted [S, D], PE transpose ->
         ctxTT, out = ctxTT.T @ Wo + bo. t-tiles 0:2 only need head
         group 0, so they start before group 1 finishes.

  Emission interleave: P1(b+1) fills the PE during P2(b); tail(0)/P3(0)
  and tail(1)-group0 fill DVE/PE during P2(1).
"""
import numpy as np

import concourse.bass as bass
import concourse.tile as tile
from concourse import mybir, bacc
from concourse import bass_utils
from concourse.masks import make_identity

f32 = mybir.dt.float32
bf16 = mybir.dt.bfloat16
AF = mybir.ActivationFunctionType
OP = mybir.AluOpType

B, S, D = 16, 577, 768
H, Dh = 12, 64
N_CORES = 8
BL = B // N_CORES
NK = D // 128              # 6 contraction chunks
NT = (S + 127) // 128      # 5 seq tiles
LAST = S - 4 * 128         # 65
SQ = 578
EPS = 1e-5
GN_N = float(Dh * S)
KW = [128, 128, 128, 128, LAST]


def bcast_ap(row_ap, nrows):
    """Partition-broadcast AP: repeat a single-partition row over nrows."""
    return bass.AP(tensor=row_ap.tensor, offset=row_ap.offset,
                   ap=[list(row_ap.ap[0]), [0, nrows]] + [list(x) for x in row_ap.ap[1:]])


def build_program(lam: float):
    nc = bacc.Bacc(trn_type="TRN2", target_bir_lowering=False, debug=False)

    x = nc.dram_tensor("x", [BL, S, D], f32, kind="ExternalInput").ap()
    Wq = nc.dram_tensor("Wq", [D, 2 * D], f32, kind="ExternalInput").ap()
    bq = nc.dram_tensor("bq", [2 * D], f32, kind="ExternalInput").ap()
    Wk = nc.dram_tensor("Wk", [D, 2 * D], f32, kind="ExternalInput").ap()
    bk = nc.dram_tensor("bk", [2 * D], f32, kind="ExternalInput").ap()
    Wv = nc.dram_tensor("Wv", [D, D], f32, kind="ExternalInput").ap()
    bv = nc.dram_tensor("bv", [D], f32, kind="ExternalInput").ap()
    Wo = nc.dram_tensor("Wo", [D, D], f32, kind="ExternalInput").ap()
    bo = nc.dram_tensor("bo", [D], f32, kind="ExternalInput").ap()
    gn_w = nc.dram_tensor("gn_w", [D], f32, kind="ExternalInput").ap()
    gn_b = nc.dram_tensor("gn_b", [D], f32, kind="ExternalInput").ap()
    out = nc.dram_tensor("out", [BL, S, D], f32, kind="ExternalOutput").ap()

    with tile.TileContext(nc) as tc:
        build_body(nc, tc, x, Wq, bq, Wk, bk, Wv, bv, Wo, bo, gn_w, gn_b, out, lam)
    nc.compile()
    return nc


def build_body(nc, tc, x, Wq, bq, Wk, bk, Wv, bv, Wo, bo, gn_w, gn_b, out, lam):
    sing = tc.alloc_tile_pool(name="sing", bufs=1)
    big = tc.alloc_tile_pool(name="big", bufs=1)
    wqk = tc.alloc_tile_pool(name="wqk", bufs=1)
    xpool = tc.alloc_tile_pool(name="xpool", bufs=2)
    epool = tc.alloc_tile_pool(name="epool", bufs=2)
    cpool = tc.alloc_tile_pool(name="cpool", bufs=12)
    tpool = tc.alloc_tile_pool(name="tpool", bufs=1)
    rpool = tc.alloc_tile_pool(name="rpool", bufs=2)
    spool = tc.alloc_tile_pool(name="spool", bufs=1)
    drpool = tc.alloc_tile_pool(name="drpool", bufs=1, space="DRAM")
    ps = tc.alloc_tile_pool(name="ps", bufs=1, space="PSUM")

    # "sc" slots (2 banks x 2 bufs): exclusively the score matmuls, so the
    # exp cadence never stalls on interleaved projection work.
    def sc_tile(name, shape=(128, 768), dtype=f32):
        return ps.tile(list(shape), dtype, tag="sc", bufs=2, name=name,
                       padded_shape=None)

    # ctx accumulators + everything else (projections, V, transposes, out,
    # GN reduce) share the other 2x2-bank ring.
    def ctx_tile(name):
        return ps.tile([65, 640], f32, tag="ctx", bufs=2, name=name)

    def aux_tile(name, shape=(128, 768), dtype=f32):
        return ps.tile(list(shape), dtype, tag="ctx", bufs=2, name=name)

    # ---------------- singles ----------------
    ones64 = sing.tile([64, 1], f32, tag="ones64", name="ones64")
    nc.gpsimd.memset(ones64, 1.0)
    onesrow = sing.tile([1, 128], bf16, tag="onesrow", name="onesrow")
    nc.gpsimd.memset(onesrow, 1.0)
    eps_t = sing.tile([1, 1], f32, tag="eps_t", name="eps_t")
    nc.gpsimd.memset(eps_t, EPS)
    ident = sing.tile([128, 128], bf16, tag="ident", name="ident")
    make_identity(nc, ident)

    # head-paired biases: bqT12[p, h] = bq[64h+p] (p<64) | bq[D+64h+p-64]
    bqT12 = sing.tile([128, H], f32, tag="bqT12", name="bqT12")
    bkT12 = sing.tile([128, H], f32, tag="bkT12", name="bkT12")
    for bt, src in ((bqT12, bq), (bkT12, bk)):
        nc.sync.dma_start(out=bt[0:64, :],
                          in_=bass.AP(tensor=src.tensor, offset=src.offset,
                                      ap=[[1, 64], [64, H]]))
        nc.sync.dma_start(out=bt[64:128, :],
                          in_=bass.AP(tensor=src.tensor, offset=src.offset + D,
                                      ap=[[1, 64], [64, H]]))
    gn_wT = sing.tile([64, H], f32, tag="gn_wT", name="gn_wT")
    nc.sync.dma_start(out=gn_wT, in_=bass.AP(tensor=gn_w.tensor, offset=gn_w.offset,
                                             ap=[[1, 64], [64, H]]))
    gn_bT = sing.tile([64, H], f32, tag="gn_bT", name="gn_bT")
    nc.sync.dma_start(out=gn_bT, in_=bass.AP(tensor=gn_b.tensor, offset=gn_b.offset,
                                             ap=[[1, 64], [64, H]]))

    # bias rows -> bf16
    bvo16 = sing.tile([1, 2 * D], bf16, tag="bvo16", name="bvo16")
    for i, src in enumerate((bv, bo)):
        bt = xpool.tile([1, D], f32, tag="xn", bufs=1, name=f"bt{i}")
        nc.gpsimd.dma_start(out=bt,
                            in_=bass.AP(tensor=src.tensor, offset=src.offset,
                                        ap=[[D, 1], [1, D]]))
        nc.vector.tensor_copy(bvo16[0:1, i * D:(i + 1) * D], bt)
    bvb = bvo16[0:1, 0:D]
    bob = bvo16[0:1, D:2 * D]

    # Wv / Wo resident bf16; Wq / Wk -> bf16 DRAM scratch, head-paired
    # [k, p, h, side, 64]. Wq first (unblocks batch-0 Q-proj), Wo last.
    WvB = sing.tile([128, NK, D], bf16, tag="WvB", name="WvB")
    WoB = sing.tile([128, NK, D], bf16, tag="WoB", name="WoB")
    WqS = sing.tile([128, NK, H, 2, 64], bf16, tag="WqS", name="WqS")
    WkB = drpool.tile([NK, 128, H, 2, 64], bf16, tag="WkB", name="WkB")

    def emit_w_prep():
        # full-row [128, 1536] f32 staging chunks stream at DMA bandwidth
        # (ring of 2 "ot" slots); loads stay off the ACT queue (exp lives
        # there). Wq casts land directly in resident WqS; Wk goes through a
        # bf16 staging cast to the DRAM scratch.
        def wq_res_prep():
            for k in range(NK):
                wt = xpool.tile([128, 2 * D], f32, tag="ot", name=f"wqr_{k}")
                nc.sync.dma_start(out=wt, in_=Wq[k * 128:(k + 1) * 128, :])
                nc.vector.tensor_copy(WqS[:, k],
                                      wt.rearrange("p (s h c) -> p h s c", s=2, h=H))

        def wk_prep():
            for k in range(NK):
                wt = xpool.tile([128, 2 * D], f32, tag="ot", name=f"wk_{k}")
                nc.sync.dma_start(out=wt, in_=Wk[k * 128:(k + 1) * 128, :])
                wc = xpool.tile([128, 2 * D], bf16, tag="xb", name=f"wkc_{k}")
                nc.vector.tensor_copy(wc, wt)
                for s in range(2):
                    nc.gpsimd.dma_start(
                        out=WkB[k][:, :, s, :],
                        in_=wc[:, s * D:(s + 1) * D].rearrange("p (h c) -> p h c", h=H))

        def vo_prep(dstW, srcW, tagn):
            for k in range(0, NK, 2):
                wt = xpool.tile([128, 2 * D], f32, tag="ot", name=f"w{tagn}_{k}")
                nc.sync.dma_start(
                    out=wt,
                    in_=bass.AP(tensor=srcW.tensor,
                                offset=srcW.offset + k * 128 * D,
                                ap=[[D, 128], [128 * D, 2], [1, D]]))
                nc.vector.tensor_copy(dstW[:, k, :], wt[:, 0:D])
                nc.vector.tensor_copy(dstW[:, k + 1, :], wt[:, D:2 * D])

        return (wq_res_prep, lambda: vo_prep(WvB, Wv, "v"),
                wk_prep, lambda: vo_prep(WoB, Wo, "o"))

    # per-batch persistent tiles
    xT = [big.tile([128, NK, 640], bf16, tag="xT", bufs=1, name=f"xT{b}") for b in range(BL)]
    Q12 = [big.tile([128, H, SQ], bf16, tag=f"Q12_{b}", name=f"Q12_{b}") for b in range(BL)]
    K12 = [big.tile([128, H, SQ], bf16, tag=f"K12_{b}", name=f"K12_{b}") for b in range(BL)]
    vaug = [big.tile([128, NT, H, 65], bf16, tag=f"vaug{b}", name=f"vaug{b}") for b in range(BL)]
    ctxf = [big.tile([65, H, SQ], bf16, tag=f"ctxf{b}", name=f"ctxf{b}") for b in range(BL)]
    # den_all quadrants (32-aligned for DVE partition-base rules):
    # head group g = h // 6, side s: row = 64*g + 32*s + (h % 6)
    den_all = [spool.tile([102, SQ], bf16, tag=f"den{b}", name=f"den{b}") for b in range(BL)]
    stats = [spool.tile([64, 2 * H], f32, tag=f"stats{b}", name=f"stats{b}") for b in range(BL)]
    csh2 = [[None] * H for _ in range(BL)]
    r16 = [spool.tile([102, SQ], bf16, tag=f"r16_{b}", name=f"r16_{b}") for b in range(BL)]
    scr = [drpool.tile([608, D], bf16, tag=f"scr{b}", name=f"scr{b}") for b in range(BL)]

    for b in range(BL):
        nc.gpsimd.memset(vaug[b][:, 0:NT - 1, :, 64:65], 1.0)
        nc.gpsimd.memset(vaug[b][0:LAST, NT - 1, :, 64:65], 1.0)

    # zero-fill scratch pad rows (577:608) so P3 transposes read finite data
    zpad = xpool.tile([128, D], bf16, tag="xb", name="zpad")
    nc.vector.memset(zpad, 0.0)
    for b in range(BL):
        nc.gpsimd.dma_start(out=scr[b][S:608, :], in_=zpad[0:608 - S, :])

    # ---------------- phase emitters ----------------
    def p1_thunks(b):
        th = []

        def x_thunk(t):
            def f():
                sz = 128 if t < NT - 1 else LAST
                xn = xpool.tile([128, D], f32, tag="xn", bufs=1, name=f"xn{b}_{t}")
                nc.gpsimd.dma_start(out=xn[0:sz, :], in_=x[b, t * 128:t * 128 + sz, :])
                xb = xpool.tile([128, D], bf16, tag="xb", name=f"xb{b}_{t}")
                if sz < 128:
                    nc.vector.memset(xb, 0.0)
                nc.vector.tensor_copy(xb[0:sz, :], xn[0:sz, :])
                tp = aux_tile(f"tpx{b}_{t}", (128, 1536), bf16)
                for k in range(NK):
                    nc.tensor.transpose(tp[:, k * 128:(k + 1) * 128],
                                        xb[:, k * 128:(k + 1) * 128], ident)
                nc.vector.tensor_copy(
                    xT[b][:, 0:NK, t * 128:(t + 1) * 128],
                    tp[:, 0:768].rearrange("p (k c) -> p k c", k=NK))
            return f

        def qk_thunk(h, WB, dstT, biasT, nm):
            def f():
                if nm == "q":
                    wslc = [WqS[:, k, h] for k in range(NK)]
                else:
                    wq = wqk.tile([128, NK, 2, 64], bf16, tag="wqk", name=f"w{nm}{b}_{h}")
                    nc.gpsimd.dma_start(
                        out=wq,
                        in_=bass.AP(tensor=WB.tensor, offset=WB.offset + h * 128,
                                    ap=[[H * 128, 128], [128 * H * 128, NK], [1, 128]]))
                    wslc = [wq[:, k] for k in range(NK)]
                q_ps = aux_tile(f"ps{nm}{b}_{h}")
                for k in range(NK):
                    nc.tensor.matmul(q_ps[:, 0:512], wslc[k], xT[b][:, k, 0:512],
                                     start=(k == 0), stop=(k == NK - 1),
                                     skip_group_check=True)
                    nc.tensor.matmul(q_ps[:, 512:577], wslc[k], xT[b][:, k, 512:577],
                                     start=(k == 0), stop=(k == NK - 1),
                                     skip_group_check=True)
                nc.vector.tensor_scalar(out=dstT[b][:, h, 0:577], in0=q_ps[:, 0:577],
                                        scalar1=biasT[:, h:h + 1], scalar2=None,
                                        op0=OP.add)
            return f

        def v_thunk(t):
            def f():
                sz = 128 if t < NT - 1 else LAST
                v_ps = aux_tile(f"psv{b}_{t}")
                for k in range(NK):
                    nc.tensor.matmul(v_ps[:, 0:512], xT[b][:, k, t * 128:(t + 1) * 128],
                                     WvB[:, k, 0:512], start=(k == 0), stop=False,
                                     skip_group_check=True)
                    nc.tensor.matmul(v_ps[:, 512:768], xT[b][:, k, t * 128:(t + 1) * 128],
                                     WvB[:, k, 512:768], start=(k == 0), stop=False,
                                     skip_group_check=True)
                nc.tensor.matmul(v_ps[:, 0:512], onesrow, bvb[0:1, 0:512],
                                 start=False, stop=True, skip_group_check=True)
                nc.tensor.matmul(v_ps[:, 512:768], onesrow, bvb[0:1, 512:768],
                                 start=False, stop=True, skip_group_check=True)
                nc.vector.tensor_copy(vaug[b][0:sz, t, 0:6, 0:64],
                                      v_ps[0:sz, 0:384].rearrange("p (h d) -> p h d", h=6))
                nc.vector.tensor_copy(vaug[b][0:sz, t, 6:12, 0:64],
                                      v_ps[0:sz, 384:768].rearrange("p (h d) -> p h d", h=6))
            return f

        xs = [x_thunk(t) for t in range(NT)]
        qs_ = [qk_thunk(h, None, Q12, bqT12, "q") for h in range(H)]
        ks_ = [qk_thunk(h, WkB, K12, bkT12, "k") for h in range(H)]
        vs_ = [v_thunk(t) for t in range(NT)]
        return xs, qs_, ks_, vs_

    def attn_thunks(b):
        th = []

        def head_thunk(h):
            def f():
                c1 = ctx_tile(f"c1_{b}_{h}")
                c2 = ctx_tile(f"c2_{b}_{h}")
                for kp in range(NT):
                    kw = KW[kp]
                    ksl = slice(kp * 128, kp * 128 + kw)
                    e = epool.tile([128, 2, SQ], bf16, tag="e", name=f"e{b}_{h}_{kp}")
                    ss = []
                    for side in range(2):
                        off = side * 64
                        s_ps = sc_tile(f"s{side}_{b}_{h}_{kp}")
                        nc.tensor.matmul(s_ps[0:kw, 0:512],
                                         K12[b][off:off + 64, h, ksl],
                                         Q12[b][off:off + 64, h, 0:512],
                                         start=True, stop=True, skip_group_check=True)
                        nc.tensor.matmul(s_ps[0:kw, 512:577],
                                         K12[b][off:off + 64, h, ksl],
                                         Q12[b][off:off + 64, h, 512:577],
                                         start=True, stop=True, skip_group_check=True)
                        ss.append(s_ps)
                    for side in range(2):
                        nc.scalar.activation(out=e[0:kw, side, 0:577],
                                             in_=ss[side][0:kw, 0:577],
                                             func=AF.Exp, scale=0.125)
                    for side, c in ((0, c1), (1, c2)):
                        nc.tensor.matmul(c[:, 0:512], vaug[b][0:kw, kp, h, :],
                                         e[0:kw, side, 0:512],
                                         start=(kp == 0), stop=False,
                                         skip_group_check=True)
                        nc.tensor.matmul(c[:, 512:577], vaug[b][0:kw, kp, h, :],
                                         e[0:kw, side, 512:577],
                                         start=(kp == 0), stop=(kp == NT - 1),
                                         skip_group_check=True)
                nc.vector.tensor_copy(ctxf[b][0:65, h, 0:577], c1[0:65, 0:577])
                ch2 = cpool.tile([65, SQ], bf16, tag="csh2", name=f"csh2_{b}_{h}")
                csh2[b][h] = ch2
                nc.vector.tensor_copy(ch2[0:65, 0:577], c2[0:65, 0:577])
                g = h // 6
                if h % 3 == 2:
                    # batched side-1 denominator gather for heads h-2..h
                    r0 = 64 * g + (h % 6) - 2
                    nc.sync.dma_start(out=den_all[b][r0:r0 + 3, 0:577],
                                      in_=ctxf[b][64:65, h - 2:h + 1, 0:577])
                nc.gpsimd.dma_start(
                    out=den_all[b][64 * g + 32 + (h % 6):64 * g + 33 + (h % 6), 0:577],
                    in_=ch2[64:65, 0:577])
            return f

        for h in range(H):
            th.append(head_thunk(h))
        return th

    def tail_grp(b, g):
        """Reciprocal + combines for head group g (heads 6g..6g+5)."""
        th = []
        q0 = 64 * g

        def recip():
            r_all = spool.tile([102, SQ], f32, tag="rall", bufs=1,
                               name=f"rall{b}_{g}")
            nc.vector.reciprocal(out=r_all[q0:q0 + 6, 0:577],
                                 in_=den_all[b][q0:q0 + 6, 0:577])
            nc.vector.reciprocal(out=r_all[q0 + 32:q0 + 38, 0:577],
                                 in_=den_all[b][q0 + 32:q0 + 38, 0:577])
            nc.vector.tensor_copy(r16[b][q0:q0 + 6, 0:577], r_all[q0:q0 + 6, 0:577])
            nc.vector.tensor_scalar(out=r16[b][q0 + 32:q0 + 38, 0:577],
                                    in0=r_all[q0 + 32:q0 + 38, 0:577],
                                    scalar1=-lam, scalar2=None, op0=OP.mult)
        th.append(recip)

        def combine(h):
            def f():
                rb = rpool.tile([64, 2, SQ], bf16, tag="rb", name=f"rb{b}_{h}")
                for side in range(2):
                    row = q0 + 32 * side + (h % 6)
                    nc.sync.dma_start(out=rb[:, side, 0:577],
                                      in_=bcast_ap(r16[b][row:row + 1, 0:577], 64))
                tmp = tpool.tile([64, SQ], bf16, tag="tmp", name=f"tmp{b}_{h}")
                ch = ctxf[b][0:64, h, 0:577]
                ch2 = csh2[b][h]
                nc.vector.tensor_tensor(out=tmp[:, 0:577], in0=ch, in1=rb[:, 0, 0:577],
                                        op=OP.mult)
                nc.vector.tensor_tensor(out=ch2[0:64, 0:577], in0=ch2[0:64, 0:577],
                                        in1=rb[:, 1, 0:577], op=OP.mult)
                nc.vector.scalar_tensor_tensor(out=ch, in0=tmp[:, 0:577], scalar=1.0,
                                               in1=ch2[0:64, 0:577],
                                               op0=OP.mult, op1=OP.add,
                                               accum_out=stats[b][:, h:h + 1])
                nc.vector.scalar_tensor_tensor(out=tmp[:, 0:577], in0=ch, scalar=1.0,
                                               in1=ch, op0=OP.mult, op1=OP.mult,
                                               accum_out=stats[b][:, H + h:H + h + 1])
            return f
        for h in range(6 * g, 6 * g + 6):
            th.append(combine(h))
        return th

    def tail_gn(b):
        """Whole-batch GroupNorm: one Ln/Exp table-switch pair per batch."""
        def gn_apply():
            sps = aux_tile(f"gn{b}", (1, 24))
            nc.tensor.matmul(sps[0:1, 0:2 * H], ones64, stats[b], start=True, stop=True,
                             skip_group_check=True)
            ssb = spool.tile([1, 2 * H], f32, tag=f"ssb{b}", name=f"ssb{b}")
            nc.vector.tensor_copy(ssb, sps[0:1, 0:2 * H])
            mu = spool.tile([1, H], f32, tag=f"mu{b}", name=f"mu{b}")
            nc.vector.tensor_scalar(out=mu, in0=ssb[0:1, 0:H], scalar1=1.0 / GN_N,
                                    scalar2=None, op0=OP.mult)
            musq = spool.tile([1, H], f32, tag=f"musq{b}", name=f"musq{b}")
            nc.vector.tensor_tensor(out=musq, in0=mu, in1=mu, op=OP.mult)
            var = spool.tile([1, H], f32, tag=f"var{b}", name=f"var{b}")
            nc.vector.scalar_tensor_tensor(out=var, in0=ssb[0:1, H:2 * H],
                                           scalar=1.0 / GN_N, in1=musq,
                                           op0=OP.mult, op1=OP.subtract)
            lnv = spool.tile([1, H], f32, tag=f"lnv{b}", name=f"lnv{b}")
            nc.scalar.activation(out=lnv, in_=var, func=AF.Ln, bias=eps_t, scale=1.0)
            rstd = spool.tile([1, H], f32, tag=f"rstd{b}", name=f"rstd{b}")
            nc.scalar.activation(out=rstd, in_=lnv, func=AF.Exp, scale=-0.5)
            mu_b = spool.tile([64, H], f32, tag=f"mu_b{b}", name=f"mu_b{b}")
            rstd_b = spool.tile([64, H], f32, tag=f"rstd_b{b}", name=f"rstd_b{b}")
            nc.sync.dma_start(out=mu_b, in_=bcast_ap(mu[0:1, :], 64))
            nc.sync.dma_start(out=rstd_b, in_=bcast_ap(rstd[0:1, :], 64))
            scale_all = spool.tile([64, H], f32, tag=f"scl{b}", name=f"scl{b}")
            nc.vector.tensor_tensor(out=scale_all, in0=rstd_b, in1=gn_wT, op=OP.mult)
            bias_all = spool.tile([64, H], f32, tag=f"bia{b}", name=f"bia{b}")
            nc.vector.scalar_tensor_tensor(out=bias_all, in0=mu_b, scalar=-1.0,
                                           in1=scale_all, op0=OP.mult, op1=OP.mult)
            nc.vector.tensor_tensor(out=bias_all, in0=bias_all, in1=gn_bT, op=OP.add)
            for h in range(H):
                nc.vector.tensor_scalar(out=ctxf[b][0:64, h, 0:577],
                                        in0=ctxf[b][0:64, h, 0:577],
                                        scalar1=scale_all[:, h:h + 1],
                                        scalar2=bias_all[:, h:h + 1],
                                        op0=OP.mult, op1=OP.add)
            nc.sync.dma_start(
                out=bass.AP(tensor=scr[b].tensor, offset=scr[b].offset,
                            ap=[[S, 64], [64 * S, H], [1, S]]),
                in_=ctxf[b][0:64, :, 0:577])
        return [gn_apply]

    def p3_thunks(b):
        cT = big.tile([128, NK, 640], bf16, tag="ctxTT", bufs=1, name=f"ctxTT{b}")

        def o_thunk(t):
            def f():
                sz = 128 if t < NT - 1 else LAST
                cn = xpool.tile([128, D], bf16, tag="xb", name=f"cn{b}_{t}")
                if sz < 128:
                    nc.vector.memset(cn, 0.0)
                nc.gpsimd.dma_start(out=cn[0:sz, :], in_=scr[b][t * 128:t * 128 + sz, :])
                tp = aux_tile(f"tpc{b}_{t}", (128, 1536), bf16)
                for k in range(NK):
                    nc.tensor.transpose(tp[:, k * 128:(k + 1) * 128],
                                        cn[:, k * 128:(k + 1) * 128], ident)
                cTt = cT[:, 0:NK, t * 128:(t + 1) * 128]
                nc.vector.tensor_copy(cTt, tp[:, 0:768].rearrange("p (k c) -> p k c", k=NK))
                o_ps = aux_tile(f"o{b}_{t}")
                for k in range(NK):
                    nc.tensor.matmul(o_ps[:, 0:512], cT[:, k, t * 128:(t + 1) * 128],
                                     WoB[:, k, 0:512], start=(k == 0), stop=False,
                                     skip_group_check=True)
                    nc.tensor.matmul(o_ps[:, 512:768], cT[:, k, t * 128:(t + 1) * 128],
                                     WoB[:, k, 512:768], start=(k == 0), stop=False,
                                     skip_group_check=True)
                nc.tensor.matmul(o_ps[:, 0:512], onesrow, bob[0:1, 0:512],
                                 start=False, stop=True, skip_group_check=True)
                nc.tensor.matmul(o_ps[:, 512:768], onesrow, bob[0:1, 512:768],
                                 start=False, stop=True, skip_group_check=True)
                ot = xpool.tile([128, D], f32, tag="ot", name=f"ot{b}_{t}")
                nc.vector.tensor_copy(ot[0:sz, :], o_ps[0:sz, 0:768])
                nc.sync.dma_start(out=out[b, t * 128:t * 128 + sz, :], in_=ot[0:sz, :])
            return f
        return [o_thunk(t) for t in range(NT)]

    def drive(primary, fillers, hook=None):
        n, m = len(primary), len(fillers)
        fi = 0
        for i, p in enumerate(primary):
            p()
            if hook is not None:
                hook(i)
            target = (i + 1) * m // n
            while fi < target:
                fillers[fi]()
                fi += 1
        while fi < m:
            fillers[fi]()
            fi += 1

    # ---------------- emission ----------------
    wq_prep, wv_prep, wk_prep, wo_prep = emit_w_prep()
    x0, q0, k0, v0 = p1_thunks(0)
    for t in x0:
        t()
    wq_prep()
    for t in q0:
        t()
    wk_prep()
    for t in k0:
        t()
    wv_prep()
    for t in v0:
        t()
    x1, q1, k1, v1 = p1_thunks(1)
    drive(attn_thunks(0), x1 + q1 + k1 + [wo_prep] + v1)

    # batch-0 tail + p3 fill attn(1); batch-1 group-0 recip/combines fire
    # mid-way so only group 1 is left at the end
    tail0 = tail_grp(0, 0) + tail_grp(0, 1) + tail_gn(0)
    p30 = p3_thunks(0)
    tail1a = tail_grp(1, 0)
    fired = [False]

    def hook(i):
        if i == 5 and not fired[0]:
            fired[0] = True
            for t in tail1a:
                t()

    drive(attn_thunks(1), tail0 + p30, hook=hook)
    if not fired[0]:
        for t in tail1a:
            t()
    for t in tail_grp(1, 1) + tail_gn(1) + p3_thunks(1):
        t()

    for p in (ps, drpool, spool, rpool, tpool, cpool, epool, xpool, wqk, big, sing):
        p.release()


_CACHE = {}
LAST_EXEC_NS = 0
LAST_TRACE = None


def _get_program(lam: float):
    key = round(float(lam), 8)
    if key not in _CACHE:
        _CACHE[key] = build_program(float(lam))
    return _CACHE[key]


def kernel(**inputs):
    x = np.ascontiguousarray(np.asarray(inputs["x"], dtype=np.float32))
    lam = float(np.asarray(inputs["lam"]))
    nc = _get_program(lam)
    names = ["Wq", "bq", "Wk", "bk", "Wv", "bv", "Wo", "bo", "gn_w", "gn_b"]
    shared = {n: np.ascontiguousarray(np.asarray(inputs[n], dtype=np.float32))
              for n in names}
    in_maps = []
    for c in range(N_CORES):
        m = dict(shared)
        m["x"] = x[c * BL:(c + 1) * BL]
        in_maps.append(m)
    res = bass_utils.run_bass_kernel_spmd(nc, in_maps, list(range(N_CORES)))
    global LAST_EXEC_NS, LAST_TRACE
    if getattr(res, "exec_time_ns", None):
        LAST_EXEC_NS = res.exec_time_ns
        LAST_TRACE = getattr(res, "instructions_and_trace", None)
    return np.concatenate([res.results[c]["out"] for c in range(N_CORES)], axis=0)


# revision 50
# speedup vs baseline: 1.0978x; 1.0978x over previous
"""Differential multi-head attention kernel for Trainium2 (8 NeuronCores).

Data-parallel over batch (16/8 = 2 per core). Per core, software-pipelined:

  init:  weights cast to bf16 once (Wq/Wk to DRAM scratch in a head-paired
         layout: head h's stationary cols are [q1|q2], so the dual-softmax
         score matmuls row-pack into PE array halves 0:64 / 64:128 and run
         concurrently). Wq loads go first so batch-0 Q-proj starts early.
  P1(b): x -> bf16 -> PE transpose -> xT; Q/K projections write Q12/K12
         (head h: side1 on partitions 0:64, side2 on 64:128); V -> vaug
         (ones col 64 makes the softmax denominators fall out of ctx MMs).
  P2(b): per head: row-packed score MMs into a merged [128,2,1024] psum,
         ONE exp per (h,kp) for both sides (ACT, bf16 out), ctx MMs
         accumulate [65, S]; ctx drained on DVE (row 64 = denominators,
         gathered into 32-aligned quadrants of den_all).
  tail(b), per 6-head group: reciprocals, -lam fold, bf16 broadcast,
         combines on DVE (stats via accum_out), per-group GroupNorm
         (rstd = exp(-0.5 ln(var+eps)) keeps ACT on one table set),
         apply, per-group scratch write.
  P3(b): read the bf16 scratch reinterpreted [S, D], PE transpose ->
         ctxTT, out = ctxTT.T @ Wo + bo. t-tiles 0:2 only need head
         group 0, so they start before group 1 finishes.

  Emission interleave: P1(b+1) fills the PE during P2(b); tail(0)/P3(0)
  and tail(1)-group0 fill DVE/PE during P2(1).
"""
import numpy as np

import concourse.bass as bass
import concourse.tile as tile
from concourse import mybir, bacc
from concourse import bass_utils
from concourse.masks import make_identity

f32 = mybir.dt.float32
bf16 = mybir.dt.bfloat16
AF = mybir.ActivationFunctionType
OP = mybir.AluOpType

B, S, D = 16, 577, 768
H, Dh = 12, 64
N_CORES = 8
BL = B // N_CORES
NK = D // 128              # 6 contraction chunks
NT = (S + 127) // 128      # 5 seq tiles
LAST = S - 4 * 128         # 65
SQ = 578
EPS = 1e-5
GN_N = float(Dh * S)
KW = [128, 128, 128, 128, LAST]


def bcast_ap(row_ap, nrows):
    """Partition-broadcast AP: repeat a single-partition row over nrows."""
    return bass.AP(tensor=row_ap.tensor, offset=row_ap.offset,
                   ap=[list(row_ap.ap[0]), [0, nrows]] + [list(x) for x in row_ap.ap[1:]])


def build_program(lam: float):
    nc = bacc.Bacc(trn_type="TRN2", target_bir_lowering=False, debug=False)

    x = nc.dram_tensor("x", [BL, S, D], f32, kind="ExternalInput").ap()
    Wq = nc.dram_tensor("Wq", [D, 2 * D], f32, kind="ExternalInput").ap()
    bq = nc.dram_tensor("bq", [2 * D], f32, kind="ExternalInput").ap()
    Wk = nc.dram_tensor("Wk", [D, 2 * D], f32, kind="ExternalInput").ap()
    bk = nc.dram_tensor("bk", [2 * D], f32, kind="ExternalInput").ap()
    Wv = nc.dram_tensor("Wv", [D, D], f32, kind="ExternalInput").ap()
    bv = nc.dram_tensor("bv", [D], f32, kind="ExternalInput").ap()
    Wo = nc.dram_tensor("Wo", [D, D], f32, kind="ExternalInput").ap()
    bo = nc.dram_tensor("bo", [D], f32, kind="ExternalInput").ap()
    gn_w = nc.dram_tensor("gn_w", [D], f32, kind="ExternalInput").ap()
    gn_b = nc.dram_tensor("gn_b", [D], f32, kind="ExternalInput").ap()
    out = nc.dram_tensor("out", [BL, S, D], f32, kind="ExternalOutput").ap()

    with tile.TileContext(nc) as tc:
        build_body(nc, tc, x, Wq, bq, Wk, bk, Wv, bv, Wo, bo, gn_w, gn_b, out, lam)
    nc.compile()
    return nc


def build_body(nc, tc, x, Wq, bq, Wk, bk, Wv, bv, Wo, bo, gn_w, gn_b, out, lam):
    sing = tc.alloc_tile_pool(name="sing", bufs=1)
    big = tc.alloc_tile_pool(name="big", bufs=1)
    wqk = tc.alloc_tile_pool(name="wqk", bufs=2)
    xpool = tc.alloc_tile_pool(name="xpool", bufs=2)
    epool = tc.alloc_tile_pool(name="epool", bufs=2)
    cpool = tc.alloc_tile_pool(name="cpool", bufs=12)
    tpool = tc.alloc_tile_pool(name="tpool", bufs=1)
    rpool = tc.alloc_tile_pool(name="rpool", bufs=2)
    spool = tc.alloc_tile_pool(name="spool", bufs=1)
    drpool = tc.alloc_tile_pool(name="drpool", bufs=1, space="DRAM")
    ps = tc.alloc_tile_pool(name="ps", bufs=1, space="PSUM")

    # "sc" slots (2 banks x 2 bufs): exclusively the score matmuls, so the
    # exp cadence never stalls on interleaved projection work.
    def sc_tile(name, shape=(128, 768), dtype=f32):
        return ps.tile(list(shape), dtype, tag="sc", bufs=2, name=name,
                       padded_shape=None)

    # ctx accumulators + everything else (projections, V, transposes, out,
    # GN reduce) share the other 2x2-bank ring.
    def ctx_tile(name):
        return ps.tile([65, 640], f32, tag="ctx", bufs=2, name=name)

    def aux_tile(name, shape=(128, 768), dtype=f32):
        return ps.tile(list(shape), dtype, tag="ctx", bufs=2, name=name)

    # ---------------- singles ----------------
    ones64 = sing.tile([64, 1], f32, tag="ones64", name="ones64")
    nc.gpsimd.memset(ones64, 1.0)
    onesrow = sing.tile([1, 128], bf16, tag="onesrow", name="onesrow")
    nc.gpsimd.memset(onesrow, 1.0)
    eps_t = sing.tile([1, 1], f32, tag="eps_t", name="eps_t")
    nc.gpsimd.memset(eps_t, EPS)
    ident = sing.tile([128, 128], bf16, tag="ident", name="ident")
    make_identity(nc, ident)

    # head-paired biases: bqT12[p, h] = bq[64h+p] (p<64) | bq[D+64h+p-64]
    bqT12 = sing.tile([128, H], f32, tag="bqT12", name="bqT12")
    bkT12 = sing.tile([128, H], f32, tag="bkT12", name="bkT12")
    for bt, src in ((bqT12, bq), (bkT12, bk)):
        nc.sync.dma_start(out=bt[0:64, :],
                          in_=bass.AP(tensor=src.tensor, offset=src.offset,
                                      ap=[[1, 64], [64, H]]))
        nc.sync.dma_start(out=bt[64:128, :],
                          in_=bass.AP(tensor=src.tensor, offset=src.offset + D,
                                      ap=[[1, 64], [64, H]]))
    gn_wT = sing.tile([64, H], f32, tag="gn_wT", name="gn_wT")
    nc.sync.dma_start(out=gn_wT, in_=bass.AP(tensor=gn_w.tensor, offset=gn_w.offset,
                                             ap=[[1, 64], [64, H]]))
    gn_bT = sing.tile([64, H], f32, tag="gn_bT", name="gn_bT")
    nc.sync.dma_start(out=gn_bT, in_=bass.AP(tensor=gn_b.tensor, offset=gn_b.offset,
                                             ap=[[1, 64], [64, H]]))

    # bias rows -> bf16
    bvo16 = sing.tile([1, 2 * D], bf16, tag="bvo16", name="bvo16")
    for i, src in enumerate((bv, bo)):
        bt = xpool.tile([1, D], f32, tag="xn", bufs=1, name=f"bt{i}")
        nc.gpsimd.dma_start(out=bt,
                            in_=bass.AP(tensor=src.tensor, offset=src.offset,
                                        ap=[[D, 1], [1, D]]))
        nc.vector.tensor_copy(bvo16[0:1, i * D:(i + 1) * D], bt)
    bvb = bvo16[0:1, 0:D]
    bob = bvo16[0:1, D:2 * D]

    # Wv / Wo resident bf16; Wq / Wk -> bf16 DRAM scratch, head-paired
    # [k, p, h, side, 64]. Wq first (unblocks batch-0 Q-proj), Wo last.
    WvB = sing.tile([128, NK, D], bf16, tag="WvB", name="WvB")
    WoB = sing.tile([128, NK, D], bf16, tag="WoB", name="WoB")
    WqS = sing.tile([128, NK, H, 2, 64], bf16, tag="WqS", name="WqS")
    WkB = drpool.tile([NK, 128, H, 2, 64], bf16, tag="WkB", name="WkB")

    def emit_w_prep():
        # full-row [128, 1536] f32 staging chunks stream at DMA bandwidth
        # (ring of 2 "ot" slots); loads stay off the ACT queue (exp lives
        # there). Wq casts land directly in resident WqS; Wk goes through a
        # bf16 staging cast to the DRAM scratch.
        def wq_res_prep():
            for k in range(NK):
                wt = xpool.tile([128, 2 * D], f32, tag="ot", name=f"wqr_{k}")
                nc.sync.dma_start(out=wt, in_=Wq[k * 128:(k + 1) * 128, :])
                nc.vector.tensor_copy(WqS[:, k],
                                      wt.rearrange("p (s h c) -> p h s c", s=2, h=H))

        def wk_prep():
            for k in range(NK):
                wt = xpool.tile([128, 2 * D], f32, tag="ot", name=f"wk_{k}")
                nc.sync.dma_start(out=wt, in_=Wk[k * 128:(k + 1) * 128, :])
                wc = xpool.tile([128, 2 * D], bf16, tag="xb", name=f"wkc_{k}")
                nc.vector.tensor_copy(wc, wt)
                for s in range(2):
                    nc.gpsimd.dma_start(
                        out=WkB[k][:, :, s, :],
                        in_=wc[:, s * D:(s + 1) * D].rearrange("p (h c) -> p h c", h=H))

        def vo_prep(dstW, srcW, tagn):
            for k in range(0, NK, 2):
                wt = xpool.tile([128, 2 * D], f32, tag="ot", name=f"w{tagn}_{k}")
                nc.sync.dma_start(
                    out=wt,
                    in_=bass.AP(tensor=srcW.tensor,
                                offset=srcW.offset + k * 128 * D,
                                ap=[[D, 128], [128 * D, 2], [1, D]]))
                nc.vector.tensor_copy(dstW[:, k, :], wt[:, 0:D])
                nc.vector.tensor_copy(dstW[:, k + 1, :], wt[:, D:2 * D])

        return (wq_res_prep, lambda: vo_prep(WvB, Wv, "v"),
                wk_prep, lambda: vo_prep(WoB, Wo, "o"))

    # per-batch persistent tiles
    xT = [big.tile([128, NK, 640], bf16, tag="xT", bufs=1, name=f"xT{b}") for b in range(BL)]
    Q12 = [big.tile([128, H, SQ], bf16, tag=f"Q12_{b}", name=f"Q12_{b}") for b in range(BL)]
    K12 = [big.tile([128, H, SQ], bf16, tag=f"K12_{b}", name=f"K12_{b}") for b in range(BL)]
    vaug = [big.tile([128, NT, H, 65], bf16, tag=f"vaug{b}", name=f"vaug{b}") for b in range(BL)]
    ctxf = [big.tile([65, H, SQ], bf16, tag=f"ctxf{b}", name=f"ctxf{b}") for b in range(BL)]
    # den_all quadrants (32-aligned for DVE partition-base rules):
    # head group g = h // 6, side s: row = 64*g + 32*s + (h % 6)
    den_all = [spool.tile([102, SQ], bf16, tag=f"den{b}", name=f"den{b}") for b in range(BL)]
    stats = [spool.tile([64, 2 * H], f32, tag=f"stats{b}", name=f"stats{b}") for b in range(BL)]
    csh2 = [[None] * H for _ in range(BL)]
    r16 = den_all  # recast in place: den rows are dead once r_all is computed
    scr = [drpool.tile([608, D], bf16, tag=f"scr{b}", name=f"scr{b}") for b in range(BL)]

    for b in range(BL):
        nc.gpsimd.memset(vaug[b][:, 0:NT - 1, :, 64:65], 1.0)
        nc.gpsimd.memset(vaug[b][0:LAST, NT - 1, :, 64:65], 1.0)

    # zero-fill scratch pad rows (577:608) so P3 transposes read finite data
    zpad = xpool.tile([128, D], bf16, tag="xb", name="zpad")
    nc.vector.memset(zpad, 0.0)
    for b in range(BL):
        nc.gpsimd.dma_start(out=scr[b][S:608, :], in_=zpad[0:608 - S, :])

    # ---------------- phase emitters ----------------
    def p1_thunks(b):
        th = []

        def x_thunk(t):
            def f():
                sz = 128 if t < NT - 1 else LAST
                xn = xpool.tile([128, D], f32, tag="xn", bufs=1, name=f"xn{b}_{t}")
                nc.gpsimd.dma_start(out=xn[0:sz, :], in_=x[b, t * 128:t * 128 + sz, :])
                xb = xpool.tile([128, D], bf16, tag="xb", name=f"xb{b}_{t}")
                if sz < 128:
                    nc.vector.memset(xb, 0.0)
                nc.vector.tensor_copy(xb[0:sz, :], xn[0:sz, :])
                tp = aux_tile(f"tpx{b}_{t}", (128, 1536), bf16)
                for k in range(NK):
                    nc.tensor.transpose(tp[:, k * 128:(k + 1) * 128],
                                        xb[:, k * 128:(k + 1) * 128], ident)
                nc.vector.tensor_copy(
                    xT[b][:, 0:NK, t * 128:(t + 1) * 128],
                    tp[:, 0:768].rearrange("p (k c) -> p k c", k=NK))
            return f

        def qk_thunk(h, WB, dstT, biasT, nm):
            def f():
                if nm == "q":
                    wslc = [WqS[:, k, h] for k in range(NK)]
                else:
                    wq = wqk.tile([128, NK, 2, 64], bf16, tag="wqk", name=f"w{nm}{b}_{h}")
                    nc.gpsimd.dma_start(
                        out=wq,
                        in_=bass.AP(tensor=WB.tensor, offset=WB.offset + h * 128,
                                    ap=[[H * 128, 128], [128 * H * 128, NK], [1, 128]]))
                    wslc = [wq[:, k] for k in range(NK)]
                q_ps = aux_tile(f"ps{nm}{b}_{h}")
                for k in range(NK):
                    nc.tensor.matmul(q_ps[:, 0:512], wslc[k], xT[b][:, k, 0:512],
                                     start=(k == 0), stop=(k == NK - 1),
                                     skip_group_check=True)
                    nc.tensor.matmul(q_ps[:, 512:577], wslc[k], xT[b][:, k, 512:577],
                                     start=(k == 0), stop=(k == NK - 1),
                                     skip_group_check=True)
                nc.vector.tensor_scalar(out=dstT[b][:, h, 0:577], in0=q_ps[:, 0:577],
                                        scalar1=biasT[:, h:h + 1], scalar2=None,
                                        op0=OP.add)
            return f

        def v_thunk(t):
            def f():
                sz = 128 if t < NT - 1 else LAST
                v_ps = aux_tile(f"psv{b}_{t}")
                for k in range(NK):
                    nc.tensor.matmul(v_ps[:, 0:512], xT[b][:, k, t * 128:(t + 1) * 128],
                                     WvB[:, k, 0:512], start=(k == 0), stop=False,
                                     skip_group_check=True)
                    nc.tensor.matmul(v_ps[:, 512:768], xT[b][:, k, t * 128:(t + 1) * 128],
                                     WvB[:, k, 512:768], start=(k == 0), stop=False,
                                     skip_group_check=True)
                nc.tensor.matmul(v_ps[:, 0:512], onesrow, bvb[0:1, 0:512],
                                 start=False, stop=True, skip_group_check=True)
                nc.tensor.matmul(v_ps[:, 512:768], onesrow, bvb[0:1, 512:768],
                                 start=False, stop=True, skip_group_check=True)
                nc.vector.tensor_copy(vaug[b][0:sz, t, 0:6, 0:64],
                                      v_ps[0:sz, 0:384].rearrange("p (h d) -> p h d", h=6))
                nc.vector.tensor_copy(vaug[b][0:sz, t, 6:12, 0:64],
                                      v_ps[0:sz, 384:768].rearrange("p (h d) -> p h d", h=6))
            return f

        xs = [x_thunk(t) for t in range(NT)]
        qs_ = [qk_thunk(h, None, Q12, bqT12, "q") for h in range(H)]
        ks_ = [qk_thunk(h, WkB, K12, bkT12, "k") for h in range(H)]
        vs_ = [v_thunk(t) for t in range(NT)]
        return xs, qs_, ks_, vs_

    def attn_thunks(b):
        th = []

        def head_thunk(h):
            def f():
                c1 = ctx_tile(f"c1_{b}_{h}")
                c2 = ctx_tile(f"c2_{b}_{h}")
                for kp in range(NT):
                    kw = KW[kp]
                    ksl = slice(kp * 128, kp * 128 + kw)
                    e = epool.tile([128, 2, SQ], bf16, tag="e", name=f"e{b}_{h}_{kp}")
                    ss = []
                    for side in range(2):
                        off = side * 64
                        s_ps = sc_tile(f"s{side}_{b}_{h}_{kp}")
                        nc.tensor.matmul(s_ps[0:kw, 0:512],
                                         K12[b][off:off + 64, h, ksl],
                                         Q12[b][off:off + 64, h, 0:512],
                                         start=True, stop=True, skip_group_check=True)
                        nc.tensor.matmul(s_ps[0:kw, 512:577],
                                         K12[b][off:off + 64, h, ksl],
                                         Q12[b][off:off + 64, h, 512:577],
                                         start=True, stop=True, skip_group_check=True)
                        ss.append(s_ps)
                    for side in range(2):
                        nc.scalar.activation(out=e[0:kw, side, 0:577],
                                             in_=ss[side][0:kw, 0:577],
                                             func=AF.Exp, scale=0.125)
                    for side, c in ((0, c1), (1, c2)):
                        nc.tensor.matmul(c[:, 0:512], vaug[b][0:kw, kp, h, :],
                                         e[0:kw, side, 0:512],
                                         start=(kp == 0), stop=False,
                                         skip_group_check=True)
                        nc.tensor.matmul(c[:, 512:577], vaug[b][0:kw, kp, h, :],
                                         e[0:kw, side, 512:577],
                                         start=(kp == 0), stop=(kp == NT - 1),
                                         skip_group_check=True)
                nc.vector.tensor_copy(ctxf[b][0:65, h, 0:577], c1[0:65, 0:577])
                ch2 = cpool.tile([65, SQ], bf16, tag="csh2", name=f"csh2_{b}_{h}")
                csh2[b][h] = ch2
                nc.vector.tensor_copy(ch2[0:65, 0:577], c2[0:65, 0:577])
                g = h // 6
                if h % 3 == 2:
                    # batched side-1 denominator gather for heads h-2..h
                    r0 = 64 * g + (h % 6) - 2
                    nc.sync.dma_start(out=den_all[b][r0:r0 + 3, 0:577],
                                      in_=ctxf[b][64:65, h - 2:h + 1, 0:577])
                nc.gpsimd.dma_start(
                    out=den_all[b][64 * g + 32 + (h % 6):64 * g + 33 + (h % 6), 0:577],
                    in_=ch2[64:65, 0:577])
            return f

        for h in range(H):
            th.append(head_thunk(h))
        return th

    def tail_grp(b, g):
        """Reciprocal + combines for head group g (heads 6g..6g+5)."""
        th = []
        q0 = 64 * g

        def recip():
            r_all = spool.tile([102, SQ], f32, tag="rall", bufs=1,
                               name=f"rall{b}_{g}")
            nc.vector.reciprocal(out=r_all[q0:q0 + 6, 0:577],
                                 in_=den_all[b][q0:q0 + 6, 0:577])
            nc.vector.reciprocal(out=r_all[q0 + 32:q0 + 38, 0:577],
                                 in_=den_all[b][q0 + 32:q0 + 38, 0:577])
            nc.vector.tensor_copy(r16[b][q0:q0 + 6, 0:577], r_all[q0:q0 + 6, 0:577])
            nc.vector.tensor_scalar(out=r16[b][q0 + 32:q0 + 38, 0:577],
                                    in0=r_all[q0 + 32:q0 + 38, 0:577],
                                    scalar1=-lam, scalar2=None, op0=OP.mult)
        th.append(recip)

        def combine(h):
            def f():
                rb = rpool.tile([64, 2, SQ], bf16, tag="rb", name=f"rb{b}_{h}")
                for side in range(2):
                    row = q0 + 32 * side + (h % 6)
                    nc.sync.dma_start(out=rb[:, side, 0:577],
                                      in_=bcast_ap(r16[b][row:row + 1, 0:577], 64))
                tmp = tpool.tile([64, SQ], bf16, tag="tmp", name=f"tmp{b}_{h}")
                ch = ctxf[b][0:64, h, 0:577]
                ch2 = csh2[b][h]
                nc.vector.tensor_tensor(out=tmp[:, 0:577], in0=ch, in1=rb[:, 0, 0:577],
                                        op=OP.mult)
                nc.vector.tensor_tensor(out=ch2[0:64, 0:577], in0=ch2[0:64, 0:577],
                                        in1=rb[:, 1, 0:577], op=OP.mult)
                nc.vector.scalar_tensor_tensor(out=ch, in0=tmp[:, 0:577], scalar=1.0,
                                               in1=ch2[0:64, 0:577],
                                               op0=OP.mult, op1=OP.add,
                                               accum_out=stats[b][:, h:h + 1])
                nc.vector.scalar_tensor_tensor(out=tmp[:, 0:577], in0=ch, scalar=1.0,
                                               in1=ch, op0=OP.mult, op1=OP.mult,
                                               accum_out=stats[b][:, H + h:H + h + 1])
            return f
        for h in range(6 * g, 6 * g + 6):
            th.append(combine(h))
        return th

    def tail_gn(b):
        """Whole-batch GroupNorm: one Ln/Exp table-switch pair per batch."""
        def gn_apply():
            sps = aux_tile(f"gn{b}", (1, 24))
            nc.tensor.matmul(sps[0:1, 0:2 * H], ones64, stats[b], start=True, stop=True,
                             skip_group_check=True)
            ssb = spool.tile([1, 2 * H], f32, tag=f"ssb{b}", name=f"ssb{b}")
            nc.vector.tensor_copy(ssb, sps[0:1, 0:2 * H])
            mu = spool.tile([1, H], f32, tag=f"mu{b}", name=f"mu{b}")
            nc.vector.tensor_scalar(out=mu, in0=ssb[0:1, 0:H], scalar1=1.0 / GN_N,
                                    scalar2=None, op0=OP.mult)
            musq = spool.tile([1, H], f32, tag=f"musq{b}", name=f"musq{b}")
            nc.vector.tensor_tensor(out=musq, in0=mu, in1=mu, op=OP.mult)
            var = spool.tile([1, H], f32, tag=f"var{b}", name=f"var{b}")
            nc.vector.scalar_tensor_tensor(out=var, in0=ssb[0:1, H:2 * H],
                                           scalar=1.0 / GN_N, in1=musq,
                                           op0=OP.mult, op1=OP.subtract)
            lnv = spool.tile([1, H], f32, tag=f"lnv{b}", name=f"lnv{b}")
            nc.scalar.activation(out=lnv, in_=var, func=AF.Ln, bias=eps_t, scale=1.0)
            rstd = spool.tile([1, H], f32, tag=f"rstd{b}", name=f"rstd{b}")
            nc.scalar.activation(out=rstd, in_=lnv, func=AF.Exp, scale=-0.5)
            mu_b = spool.tile([64, H], f32, tag=f"mu_b{b}", name=f"mu_b{b}")
            rstd_b = spool.tile([64, H], f32, tag=f"rstd_b{b}", name=f"rstd_b{b}")
            nc.sync.dma_start(out=mu_b, in_=bcast_ap(mu[0:1, :], 64))
            nc.sync.dma_start(out=rstd_b, in_=bcast_ap(rstd[0:1, :], 64))
            scale_all = spool.tile([64, H], f32, tag=f"scl{b}", name=f"scl{b}")
            nc.vector.tensor_tensor(out=scale_all, in0=rstd_b, in1=gn_wT, op=OP.mult)
            bias_all = spool.tile([64, H], f32, tag=f"bia{b}", name=f"bia{b}")
            nc.vector.scalar_tensor_tensor(out=bias_all, in0=mu_b, scalar=-1.0,
                                           in1=scale_all, op0=OP.mult, op1=OP.mult)
            nc.vector.tensor_tensor(out=bias_all, in0=bias_all, in1=gn_bT, op=OP.add)
            for h in range(H):
                nc.vector.tensor_scalar(out=ctxf[b][0:64, h, 0:577],
                                        in0=ctxf[b][0:64, h, 0:577],
                                        scalar1=scale_all[:, h:h + 1],
                                        scalar2=bias_all[:, h:h + 1],
                                        op0=OP.mult, op1=OP.add)
            nc.sync.dma_start(
                out=bass.AP(tensor=scr[b].tensor, offset=scr[b].offset,
                            ap=[[S, 64], [64 * S, H], [1, S]]),
                in_=ctxf[b][0:64, :, 0:577])
        return [gn_apply]

    def p3_thunks(b):
        cT = big.tile([128, NK, 640], bf16, tag="ctxTT", bufs=1, name=f"ctxTT{b}")

        def o_thunk(t):
            def f():
                sz = 128 if t < NT - 1 else LAST
                cn = xpool.tile([128, D], bf16, tag="xb", name=f"cn{b}_{t}")
                if sz < 128:
                    nc.vector.memset(cn, 0.0)
                nc.gpsimd.dma_start(out=cn[0:sz, :], in_=scr[b][t * 128:t * 128 + sz, :])
                tp = aux_tile(f"tpc{b}_{t}", (128, 1536), bf16)
                for k in range(NK):
                    nc.tensor.transpose(tp[:, k * 128:(k + 1) * 128],
                                        cn[:, k * 128:(k + 1) * 128], ident)
                cTt = cT[:, 0:NK, t * 128:(t + 1) * 128]
                nc.vector.tensor_copy(cTt, tp[:, 0:768].rearrange("p (k c) -> p k c", k=NK))
                o_ps = aux_tile(f"o{b}_{t}")
                for k in range(NK):
                    nc.tensor.matmul(o_ps[:, 0:512], cT[:, k, t * 128:(t + 1) * 128],
                                     WoB[:, k, 0:512], start=(k == 0), stop=False,
                                     skip_group_check=True)
                    nc.tensor.matmul(o_ps[:, 512:768], cT[:, k, t * 128:(t + 1) * 128],
                                     WoB[:, k, 512:768], start=(k == 0), stop=False,
                                     skip_group_check=True)
                nc.tensor.matmul(o_ps[:, 0:512], onesrow, bob[0:1, 0:512],
                                 start=False, stop=True, skip_group_check=True)
                nc.tensor.matmul(o_ps[:, 512:768], onesrow, bob[0:1, 512:768],
                                 start=False, stop=True, skip_group_check=True)
                ot = xpool.tile([128, D], f32, tag="ot", name=f"ot{b}_{t}")
                nc.vector.tensor_copy(ot[0:sz, :], o_ps[0:sz, 0:768])
                nc.sync.dma_start(out=out[b, t * 128:t * 128 + sz, :], in_=ot[0:sz, :])
            return f
        return [o_thunk(t) for t in range(NT)]

    def drive(primary, fillers, hook=None):
        n, m = len(primary), len(fillers)
        fi = 0
        for i, p in enumerate(primary):
            p()
            if hook is not None:
                hook(i)
            target = (i + 1) * m // n
            while fi < target:
                fillers[fi]()
                fi += 1
        while fi < m:
            fillers[fi]()
            fi += 1

    # ---------------- emission ----------------
    wq_prep, wv_prep, wk_prep, wo_prep = emit_w_prep()
    x0, q0, k0, v0 = p1_thunks(0)
    for t in x0:
        t()
    wq_prep()
    for t in q0:
        t()
    wk_prep()
    for t in k0:
        t()
    wv_prep()
    for t in v0:
        t()
    x1, q1, k1, v1 = p1_thunks(1)
    drive(attn_thunks(0), x1 + q1 + k1 + [wo_prep] + v1)

    # batch-0 tail + p3 fill attn(1); batch-1 group-0 recip/combines fire
    # mid-way so only group 1 is left at the end
    tail0 = tail_grp(0, 0) + tail_grp(0, 1) + tail_gn(0)
    p30 = p3_thunks(0)
    tail1a = tail_grp(1, 0)
    fired = [False]

    def hook(i):
        if i == 5 and not fired[0]:
            fired[0] = True
            for t in tail1a:
                t()

    drive(attn_thunks(1), tail0 + p30, hook=hook)
    if not fired[0]:
        for t in tail1a:
            t()
    for t in tail_grp(1, 1) + tail_gn(1) + p3_thunks(1):
        t()

    for p in (ps, drpool, spool, rpool, tpool, cpool, epool, xpool, wqk, big, sing):
        p.release()


_CACHE = {}
LAST_EXEC_NS = 0
LAST_TRACE = None


def _get_program(lam: float):
    key = round(float(lam), 8)
    if key not in _CACHE:
        _CACHE[key] = build_program(float(lam))
    return _CACHE[key]


def kernel(**inputs):
    x = np.ascontiguousarray(np.asarray(inputs["x"], dtype=np.float32))
    lam = float(np.asarray(inputs["lam"]))
    nc = _get_program(lam)
    names = ["Wq", "bq", "Wk", "bk", "Wv", "bv", "Wo", "bo", "gn_w", "gn_b"]
    shared = {n: np.ascontiguousarray(np.asarray(inputs[n], dtype=np.float32))
              for n in names}
    in_maps = []
    for c in range(N_CORES):
        m = dict(shared)
        m["x"] = x[c * BL:(c + 1) * BL]
        in_maps.append(m)
    res = bass_utils.run_bass_kernel_spmd(nc, in_maps, list(range(N_CORES)))
    global LAST_EXEC_NS, LAST_TRACE
    if getattr(res, "exec_time_ns", None):
        LAST_EXEC_NS = res.exec_time_ns
        LAST_TRACE = getattr(res, "instructions_and_trace", None)
    return np.concatenate([res.results[c]["out"] for c in range(N_CORES)], axis=0)
